# revision 6
# baseline (speedup 1.0000x reference)
"""Butterworth IIR (order 4) over [B=128, T=160000, 1] on 8 TRN2 NeuronCores.

Strategy: a stable IIR's impulse response decays geometrically (max pole
radius ~0.668 here), so the filter is numerically exactly (tail < 3e-23)
a 128-tap causal FIR:  y[t] = sum_{k<128} h[k] x[t-k].

Chunking time into 128-sample chunks, with X[c, m] = x[c*128 + m]:
    y[c*128 + j] = sum_m X[c, m] H0[m, j] + sum_m X[c-1, m] H1[m, j]
    H0[m, j] = h[j - m]        (0 <= j - m < 128)
    H1[m, j] = h[j - m + 128]  (0 <= j - m + 128 < 128)

On device this is two accumulating TensorE matmuls per window with the
small fixed H matrices as the stationary operand and a phase-major
(transposed) view of x as the wide moving operand (N up to 512 chunks).

Measured machine model (from ntff profiles of this kernel):
  * per-core aggregate DMA cap ~410-420GB/s; two HWDGE queues reach it
    on reads, the write phase needs gpsimd's SWDGE queue too.
  * ~17us of fixed runtime overhead per launch (engine start + iram
    ~6.5us head; 256-semaphore reset sweep + barriers ~9us tail).
  * PE at full clock does ~1.05us/sequence (LDWEIGHTS is pipelined);
    the HAM clock gate needs a few us of sustained PE activity before
    it unlocks 2.4GHz.
  * An engine's instruction stream is strictly ordered: a DMA trigger
    or copy that waits on a slow semaphore convoys everything behind
    it, and DMA completion-sem reuse paces trigger issue at queue
    speed.  So: evacuation engines carry as few triggers as possible,
    and scalar's late input triggers are woven INTO its evac stream
    (their data isn't needed until compute reaches those sequences,
    ~1.05us/seq later).

Schedule (v4): f16 I/O (10.2MB/core, stream floor ~25us; f16 rounding
costs 3.7e-4 rel err vs the 2e-2 gate).
  * h + seq0's outer window pieces + most singles/pairs on sync's
    queue; scalar takes a few early singles then returns to
    evacuation, issuing its two late input pairs between evacs.
  * ~8 warmup matmuls on the H tile right after h lands (~8.5us),
    hidden behind the input DMA ramp, so the PE clock is high when
    seq0's data arrives; no fillers inside the real stream.
  * outputs: singles for seqs 0-11 (8 on gpsimd/SWDGE, 4 on sync),
    and seqs 12-15 evacuate into one contiguous tail tile shipped as
    a single 10000B-row DMA on scalar's queue after all evac work.
  * PSUM evacuation (f32 -> f16 cast) split between scalar (ACT, w0 +
    half of w2) and vector (DVE, w1 + half of w2).

Sharding: pure data-parallel, batch 128 -> 16 sequences per core.
"""

import numpy as np

B_FULL = 128
T_FULL = 160000
N_CORES = 8
SEQ_PER_CORE = B_FULL // N_CORES  # 16
CHUNK = 128
NCHUNK = T_FULL // CHUNK  # 1250
TAPS = 128
NWIN = 512  # windows 512,512,226
SEQ_COLS = NCHUNK + 1  # 1251: col 0 is the zero predecessor chunk

TAIL_SEQS = (12, 13, 14, 15)  # shipped as one contiguous DMA on scalar
GP_OUT = (0, 2, 4, 6, 8, 9, 10, 11)  # gpsimd (SWDGE) output singles
SY_OUT = (1, 3, 5, 7)  # sync output singles

_NC_CACHE = {}


def _impulse_response(b, a, n):
    """First n samples of the IIR impulse response, computed in float64
    via the same direct-form II transposed recurrence as the reference."""
    b = np.asarray(b, np.float64)
    a = np.asarray(a, np.float64)
    bn = b / a[0]
    an = a / a[0]
    order = len(a) - 1
    z = np.zeros(order, np.float64)
    h = np.zeros(n, np.float64)
    xt = 1.0
    for t in range(n):
        yt = bn[0] * xt + z[0]
        znew = np.empty_like(z)
        znew[:-1] = z[1:] + xt * bn[1:-1] - yt * an[1:-1]
        znew[-1] = xt * bn[-1] - yt * an[-1]
        z = znew
        h[t] = yt
        xt = 0.0
    return h


def _build_h_matrices(b, a):
    h = _impulse_response(b, a, TAPS)
    m = np.arange(CHUNK)[:, None]
    j = np.arange(CHUNK)[None, :]
    d0 = j - m
    d1 = j - m + CHUNK
    H0 = np.where((d0 >= 0) & (d0 < TAPS), h[np.clip(d0, 0, TAPS - 1)], 0.0)
    H1 = np.where((d1 >= 0) & (d1 < TAPS), h[np.clip(d1, 0, TAPS - 1)], 0.0)
    return np.concatenate([H0, H1], axis=1).astype(np.float16)  # [128, 256]


def _build_nc():
    import concourse.bacc as bacc
    import concourse.mybir as mybir
    from concourse.tile import TileContext

    f32 = mybir.dt.float32
    f16 = mybir.dt.float16
    nc = bacc.Bacc()
    xt = nc.declare_dram_parameter(
        "xt", [CHUNK, SEQ_PER_CORE * SEQ_COLS], f16, isOutput=False
    )
    hh = nc.declare_dram_parameter("hh", [CHUNK, 2 * CHUNK], f16, isOutput=False)
    yt = nc.declare_dram_parameter(
        "yt", [CHUNK, SEQ_PER_CORE * NCHUNK], f16, isOutput=True
    )

    wins = list(range(0, NCHUNK, NWIN))  # [0, 512, 1024]

    with TileContext(nc) as tc:
        with (
            tc.tile_pool(name="const", bufs=1) as cpool,
            tc.tile_pool(name="yout", bufs=1) as ypool,
            tc.tile_pool(name="acc", bufs=8, space="PSUM") as pspool,
        ):
            h_tile = cpool.tile([CHUNK, 2 * CHUNK], f16, name="h_tile")
            x_tile = cpool.tile(
                [CHUNK, SEQ_PER_CORE * SEQ_COLS], f16, name="x_tile"
            )
            S, A, G = nc.sync, nc.scalar, nc.gpsimd

            def in_seqs(eng, s_lo, s_hi):
                lo, hi = s_lo * SEQ_COLS, s_hi * SEQ_COLS
                eng.dma_start(out=x_tile[:, lo:hi], in_=xt[:, lo:hi])

            # --- early input triggers -------------------------------
            # h first on sync so the warmup matmuls can start ~8.5us.
            S.dma_start(out=h_tile[:], in_=hh[:])
            cuts = [0] + [w + min(NWIN, NCHUNK - w) + 1 for w in wins]
            # seq0 pieces: outer two on sync, middle on scalar
            for eng, (lo, hi) in zip([S, A, S], zip(cuts[:-1], cuts[1:])):
                eng.dma_start(out=x_tile[:, lo:hi], in_=xt[:, lo:hi])
            in_seqs(A, 1, 2)
            in_seqs(S, 2, 3)
            in_seqs(A, 3, 4)
            in_seqs(S, 4, 5)
            in_seqs(A, 5, 6)
            in_seqs(S, 6, 8)
            in_seqs(S, 10, 12)
            in_seqs(S, 14, 16)
            # scalar's in(8,9) and in(12,13) are woven into its evac
            # stream below (data needed only when compute gets there).

            # --- PE warmup ------------------------------------------
            warm = pspool.tile([CHUNK, NWIN], f32, name="p")
            for _ in range(8):
                nc.tensor.matmul(
                    warm[:, : 2 * CHUNK],
                    h_tile[:, 0:CHUNK],
                    h_tile[:],
                    start=True,
                    stop=True,
                )

            # --- output tiles ---------------------------------------
            y_tiles = {}
            for s in range(SEQ_PER_CORE):
                if s not in TAIL_SEQS:
                    y_tiles[s] = ypool.tile([CHUNK, NCHUNK], f16, name=f"y{s}")
            tail_tile = ypool.tile(
                [CHUNK, len(TAIL_SEQS) * NCHUNK], f16, name="ytail"
            )
            for s in TAIL_SEQS:
                y_tiles[s] = None  # marker; uses tail_tile with offset

            # --- main pipeline --------------------------------------
            for s in range(SEQ_PER_CORE):
                base = s * SEQ_COLS
                if s in TAIL_SEQS:
                    y_tile = tail_tile
                    yoff = (s - TAIL_SEQS[0]) * NCHUNK
                else:
                    y_tile = y_tiles[s]
                    yoff = 0
                for wi, w in enumerate(wins):
                    n = min(NWIN, NCHUNK - w)
                    p = pspool.tile([CHUNK, NWIN], f32, name="p")
                    nc.tensor.matmul(
                        p[:, :n],
                        h_tile[:, 0:CHUNK],
                        x_tile[:, base + w + 1 : base + w + 1 + n],
                        start=True,
                        stop=False,
                    )
                    nc.tensor.matmul(
                        p[:, :n],
                        h_tile[:, CHUNK : 2 * CHUNK],
                        x_tile[:, base + w : base + w + n],
                        start=False,
                        stop=True,
                    )
                    # evacuate immediately (f32 -> f16 cast): ACT takes w0,
                    # DVE w1, and they split the short last window.
                    o = yoff + w
                    if wi == 0:
                        nc.scalar.copy(out=y_tile[:, o : o + n], in_=p[:, :n])
                    elif wi == 1:
                        nc.vector.tensor_copy(out=y_tile[:, o : o + n], in_=p[:, :n])
                    else:
                        h2 = n // 2
                        nc.scalar.copy(out=y_tile[:, o : o + h2], in_=p[:, :h2])
                        nc.vector.tensor_copy(
                            out=y_tile[:, o + h2 : o + n], in_=p[:, h2:n]
                        )
                # woven late input triggers on scalar (execute between
                # evac ops; their completion-sem reuse waits are long met)
                if s == 0:
                    in_seqs(A, 8, 10)
                elif s == 2:
                    in_seqs(A, 12, 14)
                if s in GP_OUT or s in SY_OUT:
                    out_eng = G if s in GP_OUT else S
                    out_eng.dma_start(
                        out=yt[:, s * NCHUNK : (s + 1) * NCHUNK], in_=y_tiles[s][:]
                    )
            # contiguous 4-seq tail: one 10000B-row DMA on scalar's queue,
            # emitted after every evac so it can never convoy them.
            A.dma_start(
                out=yt[:, TAIL_SEQS[0] * NCHUNK :], in_=tail_tile[:]
            )
    nc.compile()
    return nc


def _run_on_device(in_maps, trace=False):
    from concourse.bass_utils import run_bass_kernel_spmd

    if "nc" not in _NC_CACHE:
        _NC_CACHE["nc"] = _build_nc()
    return run_bass_kernel_spmd(
        _NC_CACHE["nc"], in_maps, core_ids=list(range(N_CORES)), trace=trace
    )


def _prepare_in_maps(x, b, a):
    hh = _build_h_matrices(b, a)
    xs = np.ascontiguousarray(np.asarray(x, np.float32).reshape(B_FULL, T_FULL))
    in_maps = []
    for c in range(N_CORES):
        xc = xs[c * SEQ_PER_CORE : (c + 1) * SEQ_PER_CORE]
        # phase-major: xt[p, s*1251 + 1 + c'] = x[s, c'*128 + p]; col 0 of
        # each sequence block is zeros (the "previous chunk" of chunk 0).
        xt = np.zeros((CHUNK, SEQ_PER_CORE, SEQ_COLS), np.float16)
        xt[:, :, 1:] = xc.reshape(SEQ_PER_CORE, NCHUNK, CHUNK).transpose(2, 0, 1)
        in_maps.append({"xt": np.ascontiguousarray(xt.reshape(CHUNK, -1)), "hh": hh})
    return in_maps


def _assemble_output(results):
    out = np.empty((B_FULL, T_FULL, 1), np.float32)
    for c in range(N_CORES):
        ytc = np.asarray(results[c]["yt"]).reshape(CHUNK, SEQ_PER_CORE, NCHUNK)
        yc = ytc.transpose(1, 2, 0).reshape(SEQ_PER_CORE, T_FULL)
        out[c * SEQ_PER_CORE : (c + 1) * SEQ_PER_CORE, :, 0] = yc.astype(np.float32)
    return out


def kernel(x, b, a):
    in_maps = _prepare_in_maps(x, b, a)
    res = _run_on_device(in_maps, trace=False)
    return _assemble_output(res.results)


def kernel_traced(x, b, a):
    """Same as kernel() but with neuron profiling; returns (output, exec_time_ns)."""
    in_maps = _prepare_in_maps(x, b, a)
    try:
        res = _run_on_device(in_maps, trace=True)
    except ModuleNotFoundError:
        res = _run_on_device(in_maps, trace=False)
    return _assemble_output(res.results), res.exec_time_ns
